# revision 65
# baseline (speedup 1.0000x reference)
"""Trainium2 Bass kernel for nn_MemoryAugmented (scatter_memory).

Computes, for full inputs x:[64,12,883,64], M:[12,64,64]:
    score = softmax(einsum('blnd,tmd->btnm', x, M), axis=-1)
    out   = einsum('btnm,tmd->btnd', score, M)

Distribution: data-parallel over batch across 8 NeuronCores (8 batches
per core); the small memory bank M is replicated (sent pre-transformed
into two block-diagonal constant tensors so pairs of t share one
full-width 128-K matmul).

Per-core dataflow (rows r = (b, n), padded to 7*128 per batch):
  phase A  x[b,:,ntile,:] --DMA--> [P,12,64] --DVE tree-add over l-->
           xs [P,64] --PE transpose--> xsT --ACT copy x2--> xsT2 [128,512]
           (rows 0:64 and 64:128 both hold xs^T: K-replication for mm1)
  phase B  mm1: blockdiag(M[2tp]^T, M[2tp+1]^T)^T @ xsT2 -> logits
           [(2t x m)=128, 512] in PSUM; ACT exp (no max subtraction --
           |logits| < ~30, safe in fp32); mm2: exp_chunk^T @
           [blockdiag(M) | ones cols] -> [rows=128, (t0 d | t1 d | sums)]
           in PSUM; DVE reciprocal of sums + broadcast multiply
           normalizes and evacuates PSUM; one DMA per 128-row chunk
           writes out[b, :, nrange, :].
"""
import sys

for _p in ("/opt/trn_rl_repo",):
    if _p not in sys.path:
        sys.path.insert(0, _p)

from contextlib import ExitStack

import numpy as np

import concourse.bass as bass
import concourse.bacc as bacc
import concourse.tile as tile
from concourse import mybir
from concourse._compat import with_exitstack
from concourse.bass_utils import run_bass_kernel_spmd

B, L, N, D = 64, 12, 883, 64
T, MNUM = 12, 64
NCORES = 8
BS = B // NCORES          # 8 batches per core
NT = 7                    # n-tiles per batch: 6*128 + 115
G = BS * NT               # 56 row-chunks per core
NTILES = G // 4           # 14 tiles of 512 rows
F32 = mybir.dt.float32
F32R = mybir.dt.float32r
BF16 = mybir.dt.bfloat16


def build_consts(M):
    """Host-side layout prep (pure data movement) of the memory bank."""
    M = np.asarray(M, dtype=np.float32)
    mt2 = np.zeros((128, 6 * 128), np.float32)
    mbd = np.zeros((128, 6 * 130), np.float32)
    for tp in range(6):
        t0, t1 = 2 * tp, 2 * tp + 1
        mt2[0:64, tp * 128 + 0:tp * 128 + 64] = M[t0].T
        mt2[64:128, tp * 128 + 64:tp * 128 + 128] = M[t1].T
        mbd[0:64, tp * 130 + 0:tp * 130 + 64] = M[t0]
        mbd[64:128, tp * 130 + 64:tp * 130 + 128] = M[t1]
        mbd[0:64, tp * 130 + 128] = 1.0
        mbd[64:128, tp * 130 + 129] = 1.0
    eye = np.eye(128, dtype=np.float32)
    return mt2, mbd, eye


@with_exitstack
def kernel_body(ctx: ExitStack, tc: "tile.TileContext", out: bass.AP,
                x: bass.AP, mt2: bass.AP, mbd: bass.AP, eye: bass.AP):
    nc = tc.nc
    consts = ctx.enter_context(tc.tile_pool(name="consts", bufs=1))
    work = ctx.enter_context(tc.tile_pool(name="work", bufs=2))
    psum = ctx.enter_context(tc.tile_pool(name="psum", bufs=1, space="PSUM"))

    # const loads ride the scalar HWDGE ring (idle at kernel start) so the
    # first x-load isn't queued behind them on the sync ring's FIFO.
    mt2_sb = consts.tile([128, 6 * 128], F32)
    nc.scalar.dma_start(out=mt2_sb[:], in_=mt2[:])
    mbd_sb = consts.tile([128, 6 * 130], F32)
    nc.scalar.dma_start(out=mbd_sb[:], in_=mbd[:])
    eye_sb = consts.tile([128, 128], F32)
    nc.scalar.dma_start(out=eye_sb[:], in_=eye[:])
    zbias = consts.tile([128, 1], F32)
    nc.vector.memset(zbias[:], 0.0)

    for ti in range(NTILES):
        xsT = work.tile([128, 512], F32, tag="xsT", bufs=3)
        metas = []
        for c in range(4):
            g = ti * 4 + c
            b, nt = divmod(g, NT)
            n0 = nt * 128
            P = 128 if nt < NT - 1 else N - n0
            metas.append((b, n0, P))
        # two 768 KB DMAs per tile; l-sum tree runs two chunks per
        # instruction (quarter the op count of per-chunk trees, finer
        # overlap than one tile-wide load)
        for hh in range(2):
            xt = work.tile([128, 2 * L * D], F32, tag="xt", bufs=4)
            r0 = 512 * ti + 256 * hh
            nc.sync.dma_start(
                out=xt[:].rearrange("p (c f) -> p c f", c=2),
                in_=x[r0:r0 + 256, :, :]
                    .rearrange("(c p) l d -> p c (l d)", c=2),
            )
            t384 = work.tile([128, 2 * 384], F32, tag="t384", bufs=2)
            xtv = xt[:].rearrange("p (c h f) -> p c h f", c=2, h=2)
            nc.vector.tensor_add(t384[:].rearrange("p (c f) -> p c f", c=2),
                                 xtv[:, :, 0], xtv[:, :, 1])
            t192 = work.tile([128, 2 * 192], F32, tag="t192", bufs=2)
            t384v = t384[:].rearrange("p (c h f) -> p c h f", c=2, h=2)
            nc.vector.tensor_add(t192[:].rearrange("p (c f) -> p c f", c=2),
                                 t384v[:, :, 0], t384v[:, :, 1])
            t192v = t192[:].rearrange("p (c g f) -> p c g f", c=2, g=3)
            xs2 = work.tile([128, 2 * 64], F32, tag="xs2", bufs=2)
            xs2v = xs2[:].rearrange("p (c f) -> p c f", c=2)
            nc.vector.tensor_add(xs2v, t192v[:, :, 0], t192v[:, :, 1])
            xs4 = work.tile([128, 2 * 64], F32, tag="xs4", bufs=2)
            nc.vector.tensor_add(xs4[:].rearrange("p (c f) -> p c f", c=2),
                                 xs2v, t192v[:, :, 2])
            # one transpose covers both chunks of this half: xs4 is
            # [128, 2*64], so its transpose stacks chunk cc's xs^T on
            # partitions 64cc..64cc+63.
            ps_xsT = psum.tile([128, 128], F32, tag="ps_xsT", bufs=2)
            nc.tensor.transpose(ps_xsT[:], xs4[:], eye_sb[:])
            for cc in range(2):
                c = 2 * hh + cc
                cs = slice(c * 128, (c + 1) * 128)
                nc.scalar.copy(xsT[0:64, cs], ps_xsT[64 * cc:64 * cc + 64, :])
                # K-replica for the blockdiag mm1; gpsimd is otherwise idle
                # and SBUF->SBUF is legal there (PSUM is not).
                nc.gpsimd.tensor_copy(xsT[64:128, cs], xsT[0:64, cs])

        exps = []
        for tp in range(6):
            ps_log = psum.tile([128, 512], F32, tag="logits", bufs=2)
            nc.tensor.matmul(ps_log[:], mt2_sb[:, tp * 128:(tp + 1) * 128],
                             xsT[:], start=True, stop=True)
            ex = work.tile([128, 512], F32, tag="exp", bufs=12)
            nc.scalar.activation(ex[:], ps_log[:],
                                 mybir.ActivationFunctionType.Exp, bias=zbias[:])
            exps.append(ex)

        for c in range(4):
            b, n0, P = metas[c]
            ps_val = psum.tile([128, 1024], F32, tag="val", bufs=2)
            for tp in range(6):
                off = 512 * (tp // 3) + 130 * (tp % 3)
                nc.tensor.matmul(ps_val[:, off:off + 130],
                                 exps[tp][:, c * 128:(c + 1) * 128],
                                 mbd_sb[:, tp * 130:(tp + 1) * 130],
                                 start=True, stop=True)
            # sums sit at free offsets {512h + 130a + 128 + t2}; one strided
            # reciprocal covers all 12.
            sums_ap = (ps_val[:].rearrange("p (h r) -> p h r", h=2)
                       [:, :, 0:390]
                       .rearrange("p h (a r) -> p h a r", a=3)
                       [:, :, :, 128:130])
            rec = work.tile([128, 12], F32, tag="rec", bufs=4)
            nc.vector.reciprocal(
                rec[:].rearrange("p (h a t) -> p h a t", h=2, a=3), sums_ap)
            vn = work.tile([128, T * D], F32, tag="vn", bufs=10)
            for h in range(2):
                in0 = (ps_val[:, 512 * h:512 * h + 390]
                       .rearrange("p (a r) -> p a r", a=3)
                       [:, :, 0:128]
                       .rearrange("p a (t d) -> p a t d", t=2))
                in1 = (rec[:, 6 * h:6 * h + 6]
                       .rearrange("p (a t) -> p a t", a=3)
                       .unsqueeze(3)
                       .broadcast_to([128, 3, 2, D]))
                outp = (vn[:, 384 * h:384 * h + 384]
                        .rearrange("p (a t d) -> p a t d", a=3, t=2))
                nc.vector.tensor_mul(outp, in0, in1)
            # stores go out on the ACT HWDGE ring so loads (sync ring) and
            # stores generate descriptors in parallel.
            nc.scalar.dma_start(
                out=out[b, n0:n0 + P, :, :].rearrange("n t d -> n (t d)"),
                in_=vn[:P],
            )


_NC_CACHE = {}


def build_nc():
    if "nc" in _NC_CACHE:
        return _NC_CACHE["nc"]
    nc = bacc.Bacc("TRN2", target_bir_lowering=False, debug=False,
                   num_devices=NCORES)
    # x is pre-transposed on the host to [BS, N, L, D], n-padded to 896 rows
    # per batch with zeros, and flattened to [7168, 12, 64]; the output is
    # produced as [BS, N, T, D]. Per-partition DMA runs become 3 KB
    # contiguous instead of 12x256 B (descriptor-rate-bound ~175 GB/s vs
    # HBM-bound ~358 GB/s), the whole 512-row tile arrives in one DMA, and
    # every chunk is a full 128 rows so the l-sum tree runs tile-wide.
    x_ap = nc.dram_tensor("x_sh", [BS * 896, L, D], F32, kind="ExternalInput").ap()
    mt2_ap = nc.dram_tensor("mt2", [128, 6 * 128], F32, kind="ExternalInput").ap()
    mbd_ap = nc.dram_tensor("mbd", [128, 6 * 130], F32, kind="ExternalInput").ap()
    eye_ap = nc.dram_tensor("eye", [128, 128], F32, kind="ExternalInput").ap()
    out_ap = nc.dram_tensor("out", [BS, N, T, D], F32, kind="ExternalOutput").ap()
    with tile.TileContext(nc) as tc:
        kernel_body(tc, out_ap, x_ap, mt2_ap, mbd_ap, eye_ap)
    nc.compile()
    _NC_CACHE["nc"] = nc
    return nc


def make_in_maps(x, M):
    x = np.asarray(x, dtype=np.float32)
    mt2, mbd, eye = build_consts(M)
    maps = []
    for i in range(NCORES):
        xp = np.zeros((BS, 896, L, D), np.float32)
        xp[:, :N] = x[i * BS:(i + 1) * BS].transpose(0, 2, 1, 3)
        maps.append({"x_sh": xp.reshape(BS * 896, L, D),
                     "mt2": mt2, "mbd": mbd, "eye": eye})
    return maps


def kernel(x, M):
    nc = build_nc()
    in_maps = make_in_maps(x, M)
    res = run_bass_kernel_spmd(nc, in_maps, list(range(NCORES))).results
    return np.ascontiguousarray(np.concatenate(
        [res[i]["out"].transpose(0, 2, 1, 3) for i in range(NCORES)], axis=0))


if __name__ == "__main__":
    rng = np.random.default_rng(0)
    x = rng.standard_normal((B, L, N, D), dtype=np.float32)
    M = (rng.standard_normal((T, MNUM, D), dtype=np.float32) * 0.125).astype(np.float32)
    out = kernel(x, M)
    print("out", out.shape, out.dtype, float(np.abs(out).max()))


# revision 66
# speedup vs baseline: 1.0202x; 1.0202x over previous
"""Trainium2 Bass kernel for nn_MemoryAugmented (scatter_memory).

Computes, for full inputs x:[64,12,883,64], M:[12,64,64]:
    score = softmax(einsum('blnd,tmd->btnm', x, M), axis=-1)
    out   = einsum('btnm,tmd->btnd', score, M)

Distribution: data-parallel over batch across 8 NeuronCores (8 batches
per core); the small memory bank M is replicated (sent pre-transformed
into two block-diagonal constant tensors so pairs of t share one
full-width 128-K matmul).

Per-core dataflow (rows r = (b, n), padded to 7*128 per batch):
  phase A  x[b,:,ntile,:] --DMA--> [P,12,64] --DVE tree-add over l-->
           xs [P,64] --PE transpose--> xsT --ACT copy x2--> xsT2 [128,512]
           (rows 0:64 and 64:128 both hold xs^T: K-replication for mm1)
  phase B  mm1: blockdiag(M[2tp]^T, M[2tp+1]^T)^T @ xsT2 -> logits
           [(2t x m)=128, 512] in PSUM; ACT exp (no max subtraction --
           |logits| < ~30, safe in fp32); mm2: exp_chunk^T @
           [blockdiag(M) | ones cols] -> [rows=128, (t0 d | t1 d | sums)]
           in PSUM; DVE reciprocal of sums + broadcast multiply
           normalizes and evacuates PSUM; one DMA per 128-row chunk
           writes out[b, :, nrange, :].
"""
import sys

for _p in ("/opt/trn_rl_repo",):
    if _p not in sys.path:
        sys.path.insert(0, _p)

from contextlib import ExitStack

import numpy as np

import concourse.bass as bass
import concourse.bacc as bacc
import concourse.tile as tile
from concourse import mybir
from concourse._compat import with_exitstack
from concourse.bass_utils import run_bass_kernel_spmd

B, L, N, D = 64, 12, 883, 64
T, MNUM = 12, 64
NCORES = 8
BS = B // NCORES          # 8 batches per core
NT = 7                    # n-tiles per batch: 6*128 + 115
G = BS * NT               # 56 row-chunks per core
NTILES = G // 4           # 14 tiles of 512 rows
F32 = mybir.dt.float32
F32R = mybir.dt.float32r
BF16 = mybir.dt.bfloat16


def build_consts(M):
    """Host-side layout prep (pure data movement) of the memory bank."""
    M = np.asarray(M, dtype=np.float32)
    mt2 = np.zeros((128, 6 * 128), np.float32)
    mbd = np.zeros((128, 6 * 130), np.float32)
    for tp in range(6):
        t0, t1 = 2 * tp, 2 * tp + 1
        mt2[0:64, tp * 128 + 0:tp * 128 + 64] = M[t0].T
        mt2[64:128, tp * 128 + 64:tp * 128 + 128] = M[t1].T
        mbd[0:64, tp * 130 + 0:tp * 130 + 64] = M[t0]
        mbd[64:128, tp * 130 + 64:tp * 130 + 128] = M[t1]
        mbd[0:64, tp * 130 + 128] = 1.0
        mbd[64:128, tp * 130 + 129] = 1.0
    eye = np.eye(128, dtype=np.float32)
    return mt2, mbd, eye


@with_exitstack
def kernel_body(ctx: ExitStack, tc: "tile.TileContext", out: bass.AP,
                x: bass.AP, mt2: bass.AP, mbd: bass.AP, eye: bass.AP):
    nc = tc.nc
    consts = ctx.enter_context(tc.tile_pool(name="consts", bufs=1))
    work = ctx.enter_context(tc.tile_pool(name="work", bufs=2))
    psum = ctx.enter_context(tc.tile_pool(name="psum", bufs=1, space="PSUM"))

    # const loads ride the scalar HWDGE ring (idle at kernel start) so the
    # first x-load isn't queued behind them on the sync ring's FIFO.
    mt2_sb = consts.tile([128, 6 * 128], F32)
    nc.scalar.dma_start(out=mt2_sb[:], in_=mt2[:])
    mbd_sb = consts.tile([128, 6 * 130], F32)
    nc.scalar.dma_start(out=mbd_sb[:], in_=mbd[:])
    eye_sb = consts.tile([128, 128], F32)
    nc.scalar.dma_start(out=eye_sb[:], in_=eye[:])
    zbias = consts.tile([128, 1], F32)
    nc.vector.memset(zbias[:], 0.0)

    for ti in range(NTILES):
        xsT = work.tile([128, 512], F32, tag="xsT", bufs=3)
        metas = []
        for c in range(4):
            g = ti * 4 + c
            b, nt = divmod(g, NT)
            n0 = nt * 128
            P = 128 if nt < NT - 1 else N - n0
            metas.append((b, n0, P))
        # two 768 KB DMAs per tile; l-sum tree runs two chunks per
        # instruction (quarter the op count of per-chunk trees, finer
        # overlap than one tile-wide load)
        for hh in range(2):
            xt = work.tile([128, 2 * L * D], F32, tag="xt", bufs=4)
            r0 = 512 * ti + 256 * hh
            nc.sync.dma_start(
                out=xt[:].rearrange("p (c f) -> p c f", c=2),
                in_=x[r0:r0 + 256, :, :]
                    .rearrange("(c p) l d -> p c (l d)", c=2),
            )
            t384 = work.tile([128, 2 * 384], F32, tag="t384", bufs=2)
            xtv = xt[:].rearrange("p (c h f) -> p c h f", c=2, h=2)
            nc.vector.tensor_add(t384[:].rearrange("p (c f) -> p c f", c=2),
                                 xtv[:, :, 0], xtv[:, :, 1])
            t192 = work.tile([128, 2 * 192], F32, tag="t192", bufs=2)
            t384v = t384[:].rearrange("p (c h f) -> p c h f", c=2, h=2)
            nc.vector.tensor_add(t192[:].rearrange("p (c f) -> p c f", c=2),
                                 t384v[:, :, 0], t384v[:, :, 1])
            t192v = t192[:].rearrange("p (c g f) -> p c g f", c=2, g=3)
            xs2 = work.tile([128, 2 * 64], F32, tag="xs2", bufs=2)
            xs2v = xs2[:].rearrange("p (c f) -> p c f", c=2)
            nc.vector.tensor_add(xs2v, t192v[:, :, 0], t192v[:, :, 1])
            xs4 = work.tile([128, 2 * 64], F32, tag="xs4", bufs=2)
            nc.vector.tensor_add(xs4[:].rearrange("p (c f) -> p c f", c=2),
                                 xs2v, t192v[:, :, 2])
            for cc in range(2):
                c = 2 * hh + cc
                ps_xsT = psum.tile([64, 128], F32, tag="ps_xsT", bufs=2)
                nc.tensor.transpose(ps_xsT[:], xs4[:, cc * 64:(cc + 1) * 64],
                                    eye_sb[:])
                cs = slice(c * 128, (c + 1) * 128)
                nc.scalar.copy(xsT[0:64, cs], ps_xsT[:])
                # K-replica for the blockdiag mm1; gpsimd is otherwise idle
                # and SBUF->SBUF is legal there (PSUM is not).
                nc.gpsimd.tensor_copy(xsT[64:128, cs], xsT[0:64, cs])

        exps = []
        for tp in range(6):
            ps_log = psum.tile([128, 512], F32, tag="logits", bufs=2)
            nc.tensor.matmul(ps_log[:], mt2_sb[:, tp * 128:(tp + 1) * 128],
                             xsT[:], start=True, stop=True)
            ex = work.tile([128, 512], F32, tag="exp", bufs=12)
            nc.scalar.activation(ex[:], ps_log[:],
                                 mybir.ActivationFunctionType.Exp, bias=zbias[:])
            exps.append(ex)

        for c in range(4):
            b, n0, P = metas[c]
            ps_val = psum.tile([128, 1024], F32, tag="val", bufs=2)
            for tp in range(6):
                off = 512 * (tp // 3) + 130 * (tp % 3)
                nc.tensor.matmul(ps_val[:, off:off + 130],
                                 exps[tp][:, c * 128:(c + 1) * 128],
                                 mbd_sb[:, tp * 130:(tp + 1) * 130],
                                 start=True, stop=True)
            # sums sit at free offsets {512h + 130a + 128 + t2}; one strided
            # reciprocal covers all 12.
            sums_ap = (ps_val[:].rearrange("p (h r) -> p h r", h=2)
                       [:, :, 0:390]
                       .rearrange("p h (a r) -> p h a r", a=3)
                       [:, :, :, 128:130])
            rec = work.tile([128, 12], F32, tag="rec", bufs=4)
            nc.vector.reciprocal(
                rec[:].rearrange("p (h a t) -> p h a t", h=2, a=3), sums_ap)
            vn = work.tile([128, T * D], F32, tag="vn", bufs=10)
            for h in range(2):
                in0 = (ps_val[:, 512 * h:512 * h + 390]
                       .rearrange("p (a r) -> p a r", a=3)
                       [:, :, 0:128]
                       .rearrange("p a (t d) -> p a t d", t=2))
                in1 = (rec[:, 6 * h:6 * h + 6]
                       .rearrange("p (a t) -> p a t", a=3)
                       .unsqueeze(3)
                       .broadcast_to([128, 3, 2, D]))
                outp = (vn[:, 384 * h:384 * h + 384]
                        .rearrange("p (a t d) -> p a t d", a=3, t=2))
                nc.vector.tensor_mul(outp, in0, in1)
            # stores go out on the ACT HWDGE ring so loads (sync ring) and
            # stores generate descriptors in parallel.
            nc.scalar.dma_start(
                out=out[b, n0:n0 + P, :, :].rearrange("n t d -> n (t d)"),
                in_=vn[:P],
            )


_NC_CACHE = {}


def build_nc():
    if "nc" in _NC_CACHE:
        return _NC_CACHE["nc"]
    nc = bacc.Bacc("TRN2", target_bir_lowering=False, debug=False,
                   num_devices=NCORES)
    # x is pre-transposed on the host to [BS, N, L, D], n-padded to 896 rows
    # per batch with zeros, and flattened to [7168, 12, 64]; the output is
    # produced as [BS, N, T, D]. Per-partition DMA runs become 3 KB
    # contiguous instead of 12x256 B (descriptor-rate-bound ~175 GB/s vs
    # HBM-bound ~358 GB/s), the whole 512-row tile arrives in one DMA, and
    # every chunk is a full 128 rows so the l-sum tree runs tile-wide.
    x_ap = nc.dram_tensor("x_sh", [BS * 896, L, D], F32, kind="ExternalInput").ap()
    mt2_ap = nc.dram_tensor("mt2", [128, 6 * 128], F32, kind="ExternalInput").ap()
    mbd_ap = nc.dram_tensor("mbd", [128, 6 * 130], F32, kind="ExternalInput").ap()
    eye_ap = nc.dram_tensor("eye", [128, 128], F32, kind="ExternalInput").ap()
    out_ap = nc.dram_tensor("out", [BS, N, T, D], F32, kind="ExternalOutput").ap()
    with tile.TileContext(nc) as tc:
        kernel_body(tc, out_ap, x_ap, mt2_ap, mbd_ap, eye_ap)
    nc.compile()
    _NC_CACHE["nc"] = nc
    return nc


def make_in_maps(x, M):
    x = np.asarray(x, dtype=np.float32)
    mt2, mbd, eye = build_consts(M)
    maps = []
    for i in range(NCORES):
        xp = np.zeros((BS, 896, L, D), np.float32)
        xp[:, :N] = x[i * BS:(i + 1) * BS].transpose(0, 2, 1, 3)
        maps.append({"x_sh": xp.reshape(BS * 896, L, D),
                     "mt2": mt2, "mbd": mbd, "eye": eye})
    return maps


def kernel(x, M):
    nc = build_nc()
    in_maps = make_in_maps(x, M)
    res = run_bass_kernel_spmd(nc, in_maps, list(range(NCORES))).results
    return np.ascontiguousarray(np.concatenate(
        [res[i]["out"].transpose(0, 2, 1, 3) for i in range(NCORES)], axis=0))


if __name__ == "__main__":
    rng = np.random.default_rng(0)
    x = rng.standard_normal((B, L, N, D), dtype=np.float32)
    M = (rng.standard_normal((T, MNUM, D), dtype=np.float32) * 0.125).astype(np.float32)
    out = kernel(x, M)
    print("out", out.shape, out.dtype, float(np.abs(out).max()))


# revision 67
# speedup vs baseline: 1.0274x; 1.0070x over previous
"""Trainium2 Bass kernel for nn_MemoryAugmented (scatter_memory).

Computes, for full inputs x:[64,12,883,64], M:[12,64,64]:
    score = softmax(einsum('blnd,tmd->btnm', x, M), axis=-1)
    out   = einsum('btnm,tmd->btnd', score, M)

Distribution: data-parallel over batch across 8 NeuronCores (8 batches
per core); the small memory bank M is replicated (sent pre-transformed
into two block-diagonal constant tensors so pairs of t share one
full-width 128-K matmul).

Per-core dataflow (rows r = (b, n), padded to 7*128 per batch):
  phase A  x[b,:,ntile,:] --DMA--> [P,12,64] --DVE tree-add over l-->
           xs [P,64] --PE transpose--> xsT --ACT copy x2--> xsT2 [128,512]
           (rows 0:64 and 64:128 both hold xs^T: K-replication for mm1)
  phase B  mm1: blockdiag(M[2tp]^T, M[2tp+1]^T)^T @ xsT2 -> logits
           [(2t x m)=128, 512] in PSUM; ACT exp (no max subtraction --
           |logits| < ~30, safe in fp32); mm2: exp_chunk^T @
           [blockdiag(M) | ones cols] -> [rows=128, (t0 d | t1 d | sums)]
           in PSUM; DVE reciprocal of sums + broadcast multiply
           normalizes and evacuates PSUM; one DMA per 128-row chunk
           writes out[b, :, nrange, :].
"""
import sys

for _p in ("/opt/trn_rl_repo",):
    if _p not in sys.path:
        sys.path.insert(0, _p)

from contextlib import ExitStack

import numpy as np

import concourse.bass as bass
import concourse.bacc as bacc
import concourse.tile as tile
from concourse import mybir
from concourse._compat import with_exitstack
from concourse.bass_utils import run_bass_kernel_spmd

B, L, N, D = 64, 12, 883, 64
T, MNUM = 12, 64
NCORES = 8
BS = B // NCORES          # 8 batches per core
NT = 7                    # n-tiles per batch: 6*128 + 115
G = BS * NT               # 56 row-chunks per core
NTILES = G // 4           # 14 tiles of 512 rows
F32 = mybir.dt.float32
F32R = mybir.dt.float32r
BF16 = mybir.dt.bfloat16


def build_consts(M):
    """Host-side layout prep (pure data movement) of the memory bank."""
    M = np.asarray(M, dtype=np.float32)
    mt2 = np.zeros((128, 6 * 128), np.float32)
    mbd = np.zeros((128, 6 * 130), np.float32)
    for tp in range(6):
        t0, t1 = 2 * tp, 2 * tp + 1
        mt2[0:64, tp * 128 + 0:tp * 128 + 64] = M[t0].T
        mt2[64:128, tp * 128 + 64:tp * 128 + 128] = M[t1].T
        mbd[0:64, tp * 130 + 0:tp * 130 + 64] = M[t0]
        mbd[64:128, tp * 130 + 64:tp * 130 + 128] = M[t1]
        mbd[0:64, tp * 130 + 128] = 1.0
        mbd[64:128, tp * 130 + 129] = 1.0
    eye = np.eye(128, dtype=np.float32)
    return mt2, mbd, eye


@with_exitstack
def kernel_body(ctx: ExitStack, tc: "tile.TileContext", out: bass.AP,
                x: bass.AP, mt2: bass.AP, mbd: bass.AP, eye: bass.AP):
    nc = tc.nc
    consts = ctx.enter_context(tc.tile_pool(name="consts", bufs=1))
    work = ctx.enter_context(tc.tile_pool(name="work", bufs=2))
    psum = ctx.enter_context(tc.tile_pool(name="psum", bufs=1, space="PSUM"))

    # const loads ride the scalar HWDGE ring (idle at kernel start) so the
    # first x-load isn't queued behind them on the sync ring's FIFO.
    mt2_sb = consts.tile([128, 6 * 128], F32)
    nc.scalar.dma_start(out=mt2_sb[:], in_=mt2[:])
    mbd_sb = consts.tile([128, 6 * 130], F32)
    nc.scalar.dma_start(out=mbd_sb[:], in_=mbd[:])
    eye_sb = consts.tile([128, 128], F32)
    nc.scalar.dma_start(out=eye_sb[:], in_=eye[:])
    zbias = consts.tile([128, 1], F32)
    nc.vector.memset(zbias[:], 0.0)

    for ti in range(NTILES):
        xsT = work.tile([128, 512], F32, tag="xsT", bufs=3)
        metas = []
        for c in range(4):
            g = ti * 4 + c
            b, nt = divmod(g, NT)
            n0 = nt * 128
            P = 128 if nt < NT - 1 else N - n0
            metas.append((b, n0, P))
        # two 768 KB DMAs per tile; l-sum tree runs two chunks per
        # instruction (quarter the op count of per-chunk trees, finer
        # overlap than one tile-wide load)
        for hh in range(2):
            xt = work.tile([128, 2 * L * D], F32, tag="xt", bufs=4)
            r0 = 512 * ti + 256 * hh
            nc.sync.dma_start(
                out=xt[:].rearrange("p (c f) -> p c f", c=2),
                in_=x[r0:r0 + 256, :, :]
                    .rearrange("(c p) l d -> p c (l d)", c=2),
            )
            t384 = work.tile([128, 2 * 384], F32, tag="t384", bufs=2)
            xtv = xt[:].rearrange("p (c h f) -> p c h f", c=2, h=2)
            nc.vector.tensor_add(t384[:].rearrange("p (c f) -> p c f", c=2),
                                 xtv[:, :, 0], xtv[:, :, 1])
            t192 = work.tile([128, 2 * 192], F32, tag="t192", bufs=2)
            t384v = t384[:].rearrange("p (c h f) -> p c h f", c=2, h=2)
            nc.vector.tensor_add(t192[:].rearrange("p (c f) -> p c f", c=2),
                                 t384v[:, :, 0], t384v[:, :, 1])
            t192v = t192[:].rearrange("p (c g f) -> p c g f", c=2, g=3)
            xs2 = work.tile([128, 2 * 64], F32, tag="xs2", bufs=2)
            xs2v = xs2[:].rearrange("p (c f) -> p c f", c=2)
            nc.vector.tensor_add(xs2v, t192v[:, :, 0], t192v[:, :, 1])
            xs4 = work.tile([128, 2 * 64], F32, tag="xs4", bufs=2)
            nc.vector.tensor_add(xs4[:].rearrange("p (c f) -> p c f", c=2),
                                 xs2v, t192v[:, :, 2])
            for cc in range(2):
                c = 2 * hh + cc
                ps_xsT = psum.tile([64, 128], F32, tag="ps_xsT", bufs=2)
                nc.tensor.transpose(ps_xsT[:], xs4[:, cc * 64:(cc + 1) * 64],
                                    eye_sb[:])
                cs = slice(c * 128, (c + 1) * 128)
                nc.scalar.copy(xsT[0:64, cs], ps_xsT[:])
                # K-replica for the blockdiag mm1; gpsimd is otherwise idle
                # and SBUF->SBUF is legal there (PSUM is not).
                nc.gpsimd.tensor_copy(xsT[64:128, cs], xsT[0:64, cs])

        exps = []
        for tp in range(6):
            ps_log = psum.tile([128, 512], F32, tag="logits", bufs=2)
            nc.tensor.matmul(ps_log[:], mt2_sb[:, tp * 128:(tp + 1) * 128],
                             xsT[:], start=True, stop=True)
            ex = work.tile([128, 512], F32, tag="exp", bufs=16)
            nc.scalar.activation(ex[:], ps_log[:],
                                 mybir.ActivationFunctionType.Exp, bias=zbias[:])
            exps.append(ex)

        for c in range(4):
            b, n0, P = metas[c]
            ps_val = psum.tile([128, 1024], F32, tag="val", bufs=2)
            for tp in range(6):
                off = 512 * (tp // 3) + 130 * (tp % 3)
                nc.tensor.matmul(ps_val[:, off:off + 130],
                                 exps[tp][:, c * 128:(c + 1) * 128],
                                 mbd_sb[:, tp * 130:(tp + 1) * 130],
                                 start=True, stop=True)
            # sums sit at free offsets {512h + 130a + 128 + t2}; one strided
            # reciprocal covers all 12.
            sums_ap = (ps_val[:].rearrange("p (h r) -> p h r", h=2)
                       [:, :, 0:390]
                       .rearrange("p h (a r) -> p h a r", a=3)
                       [:, :, :, 128:130])
            rec = work.tile([128, 12], F32, tag="rec", bufs=4)
            nc.vector.reciprocal(
                rec[:].rearrange("p (h a t) -> p h a t", h=2, a=3), sums_ap)
            vn = work.tile([128, T * D], F32, tag="vn", bufs=10)
            for h in range(2):
                in0 = (ps_val[:, 512 * h:512 * h + 390]
                       .rearrange("p (a r) -> p a r", a=3)
                       [:, :, 0:128]
                       .rearrange("p a (t d) -> p a t d", t=2))
                in1 = (rec[:, 6 * h:6 * h + 6]
                       .rearrange("p (a t) -> p a t", a=3)
                       .unsqueeze(3)
                       .broadcast_to([128, 3, 2, D]))
                outp = (vn[:, 384 * h:384 * h + 384]
                        .rearrange("p (a t d) -> p a t d", a=3, t=2))
                nc.vector.tensor_mul(outp, in0, in1)
            # stores go out on the ACT HWDGE ring so loads (sync ring) and
            # stores generate descriptors in parallel.
            nc.scalar.dma_start(
                out=out[b, n0:n0 + P, :, :].rearrange("n t d -> n (t d)"),
                in_=vn[:P],
            )


_NC_CACHE = {}


def build_nc():
    if "nc" in _NC_CACHE:
        return _NC_CACHE["nc"]
    nc = bacc.Bacc("TRN2", target_bir_lowering=False, debug=False,
                   num_devices=NCORES)
    # x is pre-transposed on the host to [BS, N, L, D], n-padded to 896 rows
    # per batch with zeros, and flattened to [7168, 12, 64]; the output is
    # produced as [BS, N, T, D]. Per-partition DMA runs become 3 KB
    # contiguous instead of 12x256 B (descriptor-rate-bound ~175 GB/s vs
    # HBM-bound ~358 GB/s), the whole 512-row tile arrives in one DMA, and
    # every chunk is a full 128 rows so the l-sum tree runs tile-wide.
    x_ap = nc.dram_tensor("x_sh", [BS * 896, L, D], F32, kind="ExternalInput").ap()
    mt2_ap = nc.dram_tensor("mt2", [128, 6 * 128], F32, kind="ExternalInput").ap()
    mbd_ap = nc.dram_tensor("mbd", [128, 6 * 130], F32, kind="ExternalInput").ap()
    eye_ap = nc.dram_tensor("eye", [128, 128], F32, kind="ExternalInput").ap()
    out_ap = nc.dram_tensor("out", [BS, N, T, D], F32, kind="ExternalOutput").ap()
    with tile.TileContext(nc) as tc:
        kernel_body(tc, out_ap, x_ap, mt2_ap, mbd_ap, eye_ap)
    nc.compile()
    _NC_CACHE["nc"] = nc
    return nc


def make_in_maps(x, M):
    x = np.asarray(x, dtype=np.float32)
    mt2, mbd, eye = build_consts(M)
    maps = []
    for i in range(NCORES):
        xp = np.zeros((BS, 896, L, D), np.float32)
        xp[:, :N] = x[i * BS:(i + 1) * BS].transpose(0, 2, 1, 3)
        maps.append({"x_sh": xp.reshape(BS * 896, L, D),
                     "mt2": mt2, "mbd": mbd, "eye": eye})
    return maps


def kernel(x, M):
    nc = build_nc()
    in_maps = make_in_maps(x, M)
    res = run_bass_kernel_spmd(nc, in_maps, list(range(NCORES))).results
    return np.ascontiguousarray(np.concatenate(
        [res[i]["out"].transpose(0, 2, 1, 3) for i in range(NCORES)], axis=0))


if __name__ == "__main__":
    rng = np.random.default_rng(0)
    x = rng.standard_normal((B, L, N, D), dtype=np.float32)
    M = (rng.standard_normal((T, MNUM, D), dtype=np.float32) * 0.125).astype(np.float32)
    out = kernel(x, M)
    print("out", out.shape, out.dtype, float(np.abs(out).max()))
